# revision 6
# baseline (speedup 1.0000x reference)
"""Trainium2 Bass kernel for nn_DCell (hierarchical DCell-style GNN).

Sharding: subsystem-parallel across 8 NeuronCores. Each core owns 64 of the
512 leaf subsystems (16 groups of 4 leaves, block-diagonal matmuls with
K=128) and the 4 mid subsystems fed by exactly those leaves. BatchNorm batch
stats are fully local for leaf and mid layers.

Leaf BN is folded into the mid weights: leaf_out = s*tanh + t, so the mid
matmul uses weights scaled by s (per k-row) and the offset W^T t is
accumulated into the mid bias via tiny PE matmuls. This removes all
full-width BN applies from the critical path. Mid BN is likewise folded
into the root weights. Each core computes its root partial pre-activation
in a transposed wire layout [128, 16*38], summed across cores with one
AllReduce; every core then redundantly finishes the root (tanh +
full-batch BN via a gpsimd partition-reduce) and writes [128, 608] f32;
the host reassembles [2048, 38].

Leaf stats: sum(tanh) comes free from the ACT accumulator; sum(tanh^2) via
DVE bf16 square + reduce. Per-group BN folds run with a 3-group lag so the
PE never stalls on the stats chain.

kernel(**inputs) takes full unsharded inputs, returns [2048, 38] float32.
"""

import ml_dtypes
import numpy as np

import concourse.bass as bass
import concourse.mybir as mybir
import concourse.tile as tile
from concourse import bacc
from concourse import bass_utils

# Problem constants (hardcoded; kernel.py must be self-contained)
S, B, GL, OL = 512, 2048, 32, 20
M, C, GM, OM = 32, 16, 64, 20
GR, OR = 128, 38
EPS = 1e-5
NCORES = 8
LPC = S // NCORES      # 64 leaves per core
GPC = LPC // 4         # 16 leaf groups of 4 per core
MPC = M // NCORES      # 4 mids per core
BT = 512               # batch tile (free dim per matmul / psum bank)
NBT = B // BT          # 4
NCH = B // 128         # 16 batch chunks of 128 (wire layout)
LAG = 3                # groups between leaf stats and dependent mid matmuls

f32 = mybir.dt.float32
bf16 = mybir.dt.bfloat16
i32 = mybir.dt.int32
AF = mybir.ActivationFunctionType
ALU = mybir.AluOpType
AX = mybir.AxisListType
NPBF16 = ml_dtypes.bfloat16

MAGIC = 0x5F3759DF  # fast inverse sqrt seed


def _emit_rsqrt(nc, eng, sp, tag, out, a, magic_t, n, iters=2):
    """out = 1/sqrt(a) elementwise on [P, n] fp32 tiles, DVE/Pool-only.

    Quake magic seed + Newton iterations: 2 iters -> rel err ~5e-6. a > 0.
    """
    P = a.shape[0]
    sh = sp.tile([P, n], i32, tag=f"{tag}sh", name=f"{tag}sh")
    eng.tensor_scalar(sh, a.bitcast(i32), 1, None, ALU.arith_shift_right)
    y0 = sp.tile([P, n], i32, tag=f"{tag}y0", name=f"{tag}y0")
    eng.tensor_tensor(y0, magic_t[:P, 0:n], sh, ALU.subtract)
    y = y0.bitcast(f32)
    for it in range(iters):
        # y <- y * (1.5 - 0.5*a*y*y)
        t1 = sp.tile([P, n], f32, tag=f"{tag}t1", name=f"{tag}t1_{it}")
        eng.tensor_tensor(t1, y, y, ALU.mult)                # y^2
        t2 = sp.tile([P, n], f32, tag=f"{tag}t2", name=f"{tag}t2_{it}")
        eng.tensor_tensor(t2, a, t1, ALU.mult)               # a*y^2
        t3 = sp.tile([P, n], f32, tag=f"{tag}t3", name=f"{tag}t3_{it}")
        eng.tensor_scalar(t3, t2, -0.5, 1.5, ALU.mult, ALU.add)
        dst = out if it == iters - 1 else sp.tile(
            [P, n], f32, tag=f"{tag}y", name=f"{tag}y_{it}")
        eng.tensor_tensor(dst, y, t3, ALU.mult)
        y = dst


def _build_nc():
    """Build (once) the SPMD Bass program run identically on all 8 cores."""
    nc = bacc.Bacc(
        "TRN2",
        target_bir_lowering=False,
        debug=False,
        enable_asserts=False,
        num_devices=NCORES,
    )

    # ---- per-core external I/O ----
    xleaf = nc.dram_tensor("xleaf", [GPC, 128, B], bf16, kind="ExternalInput").ap()
    wleaf = nc.dram_tensor("wleaf", [128, GPC * 80], bf16, kind="ExternalInput").ap()
    bleaf = nc.dram_tensor("bleaf", [80, GPC], f32, kind="ExternalInput").ap()
    gleaf = nc.dram_tensor("gleaf", [80, GPC], f32, kind="ExternalInput").ap()
    beleaf = nc.dram_tensor("beleaf", [80, GPC], f32, kind="ExternalInput").ap()
    xmid2 = nc.dram_tensor("xmid2", [2, 128, B], bf16, kind="ExternalInput").ap()
    wgmid = nc.dram_tensor("wgmid", [80, GPC * 80], bf16, kind="ExternalInput").ap()
    wxmid2 = nc.dram_tensor("wxmid2", [128, 2 * 80], bf16, kind="ExternalInput").ap()
    bmid = nc.dram_tensor("bmid", [80, 1], f32, kind="ExternalInput").ap()
    gmid = nc.dram_tensor("gmid", [80, 1], f32, kind="ExternalInput").ap()
    bemid = nc.dram_tensor("bemid", [80, 1], f32, kind="ExternalInput").ap()
    wcroot = nc.dram_tensor("wcroot", [80, OR], bf16, kind="ExternalInput").ap()
    wgroot = nc.dram_tensor("wgroot", [16, OR], bf16, kind="ExternalInput").ap()
    xroot17 = nc.dram_tensor("xroot17", [17, B], bf16, kind="ExternalInput").ap()
    broot8 = nc.dram_tensor("broot8", [1, OR], f32, kind="ExternalInput").ap()
    grootb = nc.dram_tensor("grootb", [128, OR], f32, kind="ExternalInput").ap()
    berootb = nc.dram_tensor("berootb", [128, OR], f32, kind="ExternalInput").ap()
    y = nc.dram_tensor("y", [128, NCH * OR], f32, kind="ExternalOutput").ap()

    with tile.TileContext(nc) as tc:
        with (
            tc.tile_pool(name="const", bufs=1) as cp,
            tc.tile_pool(name="xp", bufs=4) as xp,
            tc.tile_pool(name="lt", bufs=16) as ltp,
            tc.tile_pool(name="sqp", bufs=2) as sqp,
            tc.tile_pool(name="small", bufs=2) as sp,
            tc.tile_pool(name="big", bufs=1) as bp,
            tc.tile_pool(name="psA", bufs=2, space="PSUM") as psA,
            tc.tile_pool(name="psM", bufs=4, space="PSUM") as psM,
            tc.tile_pool(name="dram", bufs=1, space="DRAM") as dp,
        ):
            # ---- load constants/weights into SBUF ----
            wleaf_sb = cp.tile_from(wleaf, forced_dma_engine=mybir.EngineType.Pool)
            bleaf_sb = cp.tile_from(bleaf, forced_dma_engine=mybir.EngineType.Pool)
            wgmid_sb = cp.tile_from(wgmid, forced_dma_engine=mybir.EngineType.Pool)
            gleaf_sb = cp.tile_from(gleaf, forced_dma_engine=mybir.EngineType.Pool)
            beleaf_sb = cp.tile_from(beleaf, forced_dma_engine=mybir.EngineType.Pool)
            wxmid2_sb = cp.tile_from(wxmid2, forced_dma_engine=mybir.EngineType.Pool)
            xmid2_sb = [
                cp.tile_from(xmid2[0], name="xmid2a",
                             forced_dma_engine=mybir.EngineType.Pool),
                cp.tile_from(xmid2[1], name="xmid2b",
                             forced_dma_engine=mybir.EngineType.Pool),
            ]
            bmid_sb = cp.tile_from(bmid, forced_dma_engine=mybir.EngineType.Pool)
            gmid_sb = cp.tile_from(gmid, forced_dma_engine=mybir.EngineType.Pool)
            bemid_sb = cp.tile_from(bemid, forced_dma_engine=mybir.EngineType.Pool)
            wcroot_sb = cp.tile_from(wcroot, forced_dma_engine=mybir.EngineType.Pool)
            broot8_sb = cp.tile_from(broot8, forced_dma_engine=mybir.EngineType.Pool)
            grootb_sb = cp.tile_from(grootb, forced_dma_engine=mybir.EngineType.Pool)
            berootb_sb = cp.tile_from(berootb, forced_dma_engine=mybir.EngineType.Pool)

            # stacked root lhsT: rows 0-79 mid tanh out, 80-96 xroot+ones
            stack = bp.tile([97, B], bf16, tag="stack", name="stack")
            nc.gpsimd.dma_start(out=stack[80:97, :], in_=xroot17)
            # stacked root weights: 0-79 scaled wcroot, 80-95 wgroot,
            # 96 = t_mid fold + b_root/8
            wrt = bp.tile([97, OR], bf16, tag="wrt", name="wrt")
            nc.gpsimd.dma_start(out=wrt[80:96, :], in_=wgroot)

            magic_t = cp.tile([80, 4], i32, tag="magic", name="magict")
            nc.vector.memset(magic_t, MAGIC)
            magic_r = cp.tile([128, OR], i32, tag="magicr", name="magicr")
            nc.vector.memset(magic_r, MAGIC)

            # scaled mid weights (written per group as folds complete)
            wgs = bp.tile([80, GPC * 80], bf16, tag="wgs", name="wgs")
            # mid bias accumulator: bmid + sum_g W_g^T t_g
            boff = bp.tile([80, 1], f32, tag="boff", name="boff")
            nc.vector.tensor_copy(boff, bmid_sb[:, :])

            # leaf stats: sums of tanh (ACT accum) and tanh^2 (DVE reduce)
            mvx = cp.tile([80, GPC, 2], f32, tag="mvx", name="mvx")
            mvq = cp.tile([80, GPC, 2], f32, tag="mvq", name="mvq")
            # per-group fold results: t (bf16) and s kept as tiles
            tbf_tiles = []
            s_tiles = []

            # persistent mid-accumulation psum banks (one per batch tile)
            mid_ps = [psM.tile([80, BT], f32, tag="mid", name=f"midps{b}")
                      for b in range(NBT)]

            lt_tiles = []

            def emit_mid_for(gi):
                """PE offset matmul + mid matmuls for (folded) group gi."""
                off_ps = psA.tile([80, 1], f32, tag="leaf",
                                  name=f"offps{gi}")
                nc.tensor.matmul(
                    off_ps[:, :],
                    wgmid_sb[:, 80 * gi:80 * gi + 80],
                    tbf_tiles[gi][:, 0:1],
                    start=True, stop=True)
                nc.vector.tensor_tensor(boff, boff, off_ps[:, :], ALU.add)
                for bt in range(NBT):
                    nc.tensor.matmul(
                        mid_ps[bt][:, :],
                        wgs[:, 80 * gi:80 * gi + 80],
                        lt_tiles[gi][:, bt * BT:(bt + 1) * BT],
                        start=False, stop=(gi == GPC - 1))

            for gi in range(GPC):
                ltile = ltp.tile([80, B], bf16, tag="lt", name=f"lt{gi}")
                lt_tiles.append(ltile)
                # ---- leaf matmul + tanh (+ ACT sum accum) per half batch ----
                xt = xp.tile([128, B], bf16, tag="x", name=f"x{gi}")
                nc.sync.dma_start(out=xt, in_=xleaf[gi])
                for h in range(2):
                    ps = psA.tile([80, 2 * BT], f32, tag="leaf",
                                  name=f"lfps{gi}_{h}")
                    for s2 in range(2):
                        nc.tensor.matmul(
                            ps[:, s2 * BT:(s2 + 1) * BT],
                            wleaf_sb[:, 80 * gi:80 * gi + 80],
                            xt[:, (2 * h + s2) * BT:(2 * h + s2 + 1) * BT],
                            start=True, stop=True)
                    nc.scalar.activation(
                        ltile[:, 2 * h * BT:2 * (h + 1) * BT], ps[:, :],
                        AF.Tanh, bias=bleaf_sb[:, gi:gi + 1], scale=1.0,
                        accum_out=mvx[:, gi, h:h + 1])
                    # sumsq via DVE bf16 square + reduce
                    sqs = sqp.tile([80, 2 * BT], bf16, tag="sqs",
                                   name=f"sqs{gi}_{h}")
                    nc.vector.tensor_tensor(
                        sqs, ltile[:, 2 * h * BT:2 * (h + 1) * BT],
                        ltile[:, 2 * h * BT:2 * (h + 1) * BT], ALU.mult)
                    nc.vector.tensor_reduce(
                        out=mvq[:, gi, h:h + 1], in_=sqs,
                        op=ALU.add, axis=AX.X)

                # ---- fold leaf BN for this group into mid weights ----
                feng = nc.vector
                sx = sp.tile([80, 1], f32, tag="sx", name=f"sx{gi}")
                feng.tensor_tensor(sx, mvx[:, gi, 0:1], mvx[:, gi, 1:2],
                                   ALU.add)
                m1 = sp.tile([80, 1], f32, tag="m1", name=f"m1{gi}")
                feng.tensor_scalar(m1, sx, 1.0 / B, None, ALU.mult)
                qx = sp.tile([80, 1], f32, tag="qx", name=f"qx{gi}")
                feng.tensor_tensor(qx, mvq[:, gi, 0:1], mvq[:, gi, 1:2],
                                   ALU.add)
                q1 = sp.tile([80, 1], f32, tag="q1", name=f"q1{gi}")
                feng.tensor_scalar(q1, qx, 1.0 / B, EPS, ALU.mult, ALU.add)
                msq = sp.tile([80, 1], f32, tag="msq", name=f"msq{gi}")
                feng.tensor_tensor(msq, m1, m1, ALU.mult)
                a1 = sp.tile([80, 1], f32, tag="a1", name=f"a1{gi}")
                feng.tensor_tensor(a1, q1, msq, ALU.subtract)  # var+eps
                rs1 = sp.tile([80, 1], f32, tag="rs1", name=f"rs1{gi}")
                _emit_rsqrt(nc, feng, sp, "lf", rs1, a1, magic_t, 1)
                s1 = sp.tile([80, 1], f32, tag="s1", name=f"s1{gi}",
                             bufs=GPC)
                feng.tensor_tensor(s1, gleaf_sb[:, gi:gi + 1], rs1, ALU.mult)
                s_tiles.append(s1)
                ms1 = sp.tile([80, 1], f32, tag="ms1", name=f"ms1{gi}")
                feng.tensor_tensor(ms1, m1, s1, ALU.mult)
                t1 = sp.tile([80, 1], f32, tag="t1f", name=f"t1f{gi}")
                feng.tensor_tensor(t1, beleaf_sb[:, gi:gi + 1], ms1,
                                   ALU.subtract)
                tbf = sp.tile([80, 1], bf16, tag="tbf", name=f"tbf{gi}",
                              bufs=GPC)
                feng.tensor_copy(tbf, t1)
                tbf_tiles.append(tbf)
                # scale this group's mid-weight block by s (per k-row)
                feng.tensor_scalar(
                    wgs[:, 80 * gi:80 * gi + 80],
                    wgmid_sb[:, 80 * gi:80 * gi + 80],
                    s1[:, 0:1], None, ALU.mult)

                # gene-input matmuls start the mid psum accumulation
                if gi == 2:
                    for bt in range(NBT):
                        for pr in range(2):
                            nc.tensor.matmul(
                                mid_ps[bt][:, :],
                                wxmid2_sb[:, 80 * pr:80 * pr + 80],
                                xmid2_sb[pr][:, bt * BT:(bt + 1) * BT],
                                start=(pr == 0), stop=False)
                # lagged mid matmuls keep PE from stalling on the fold chain
                if gi >= LAG:
                    emit_mid_for(gi - LAG)
            for gi in range(GPC - LAG, GPC):
                emit_mid_for(gi)

            # ---- mid finish: tanh into stack, BN stats, fold into root ----
            mst = sp.tile([80, NBT, 6], f32, tag="mst", name="mst")
            for bt in range(NBT):
                nc.scalar.activation(
                    stack[0:80, bt * BT:(bt + 1) * BT], mid_ps[bt][:, :],
                    AF.Tanh, bias=boff[:, 0:1], scale=1.0)
                nc.vector.bn_stats(out=mst[:, bt, :],
                                   in_=stack[0:80, bt * BT:(bt + 1) * BT])
            mmv = sp.tile([80, 2], f32, tag="mmv", name="mmv")
            nc.vector.bn_aggr(out=mmv[:, :], in_=mst[:, :, :])
            ma = sp.tile([80, 1], f32, tag="ma", name="ma")
            nc.vector.tensor_scalar_add(ma, mmv[:, 1:2], EPS)
            mrs = sp.tile([80, 1], f32, tag="mrs", name="mrs")
            _emit_rsqrt(nc, nc.vector, sp, "md", mrs, ma, magic_t, 1)
            msm = sp.tile([80, 1], f32, tag="msm", name="msm")
            nc.vector.tensor_mul(msm, gmid_sb[:, :], mrs)
            mms = sp.tile([80, 1], f32, tag="mms", name="mms")
            nc.vector.tensor_mul(mms, mmv[:, 0:1], msm)
            mtm = sp.tile([80, 1], f32, tag="mtm", name="mtm")
            nc.vector.tensor_sub(mtm, bemid_sb[:, :], mms)
            # fold mid BN: scale wcroot rows; offset row via tiny matmul
            nc.vector.tensor_scalar(wrt[0:80, :], wcroot_sb[:, :],
                                    msm[:, 0:1], None, ALU.mult)
            tmid_bf = sp.tile([80, 1], bf16, tag="tmbf", name="tmidbf")
            nc.vector.tensor_copy(tmid_bf, mtm)
            pr_ps = psA.tile([1, OR], f32, tag="leaf", name="prps")
            nc.tensor.matmul(pr_ps[:, :], tmid_bf[:, 0:1], wcroot_sb[:, :],
                             start=True, stop=True)
            nc.vector.tensor_tensor(wrt[96:97, :], pr_ps[:, :],
                                    broot8_sb[0:1, :], ALU.add)

            # ---- root partial in wire layout [128, 16*38] ----
            wire_ps = psA.tile([128, 2 * BT], f32, tag="leaf", name="wireps")
            for c in range(NCH):
                col = 38 * c if c < 13 else BT + 38 * (c - 13)
                nc.tensor.matmul(
                    wire_ps[:, col:col + OR],
                    stack[:, 128 * c:128 * (c + 1)],
                    wrt[:, :], start=True, stop=True)
            wire_sb = bp.tile([128, NCH * OR], bf16, tag="wire",
                              name="wiresb")
            nc.vector.tensor_copy(wire_sb[:, 0:13 * OR],
                                  wire_ps[:, 0:13 * OR])
            nc.vector.tensor_copy(wire_sb[:, 13 * OR:NCH * OR],
                                  wire_ps[:, BT:BT + 3 * OR])

            # ---- AllReduce the partial root pre-activation (bf16) ----
            cc_in = dp.tile([128, NCH * OR], bf16, tag="ccin", name="ccin")
            cc_out = dp.tile([128, NCH * OR], bf16, tag="ccout",
                             name="ccout", addr_space="Shared")
            nc.sync.dma_start(out=cc_in[:, :], in_=wire_sb[:, :])
            nc.gpsimd.collective_compute(
                "AllReduce",
                ALU.add,
                replica_groups=[list(range(NCORES))],
                ins=[cc_in.opt()],
                outs=[cc_out.opt()],
            )
            rsum = bp.tile([128, NCH * OR], bf16, tag="rsum", name="rsum")
            nc.sync.dma_start(out=rsum[:, :], in_=cc_out[:, :])

            # ---- root finish: tanh, full-batch BN in wire layout ----
            rt = bp.tile([128, NCH * OR], bf16, tag="rt", name="rt")
            nc.scalar.activation(rt[:, :], rsum[:, :], AF.Tanh,
                                 bias=0.0, scale=1.0)
            sq = bp.tile([128, NCH * OR], bf16, tag="rsq", name="rsq")
            nc.vector.tensor_tensor(sq, rt[:, :], rt[:, :], ALU.mult)
            rt_jc = rt[:, :].rearrange("p (c j) -> p j c", c=NCH)
            sq_jc = sq[:, :].rearrange("p (c j) -> p j c", c=NCH)
            # pack [sum, sumsq] into one [128, 2*OR] tile: one gpsimd reduce
            sq2 = sp.tile([128, 2, OR], f32, tag="sq2", name="sq2")
            nc.vector.tensor_reduce(out=sq2[:, 0, :], in_=rt_jc,
                                    op=ALU.add, axis=AX.X)
            nc.vector.tensor_reduce(out=sq2[:, 1, :], in_=sq_jc,
                                    op=ALU.add, axis=AX.X)
            import concourse.bass_isa as bass_isa
            sq2a = sp.tile([128, 2, OR], f32, tag="sq2a", name="sq2a")
            nc.gpsimd.partition_all_reduce(
                sq2a[:, :, :], sq2[:, :, :],
                channels=128, reduce_op=bass_isa.ReduceOp.add)
            mean_t = sp.tile([128, OR], f32, tag="rmean", name="rmean")
            nc.vector.tensor_scalar(mean_t, sq2a[:, 0, :], 1.0 / B, None,
                                    ALU.mult)
            msq_t = sp.tile([128, OR], f32, tag="rmsq", name="rmsq")
            nc.vector.tensor_tensor(msq_t, mean_t, mean_t, ALU.mult)
            qb_t = sp.tile([128, OR], f32, tag="rqb", name="rqb")
            nc.vector.tensor_scalar(qb_t, sq2a[:, 1, :], 1.0 / B, EPS,
                                    ALU.mult, ALU.add)
            va_t = sp.tile([128, OR], f32, tag="rva", name="rva")
            nc.vector.tensor_tensor(va_t, qb_t, msq_t, ALU.subtract)
            rrs = sp.tile([128, OR], f32, tag="rrs", name="rrs")
            _emit_rsqrt(nc, nc.vector, sp, "rt", rrs, va_t, magic_r, OR)
            rsc = sp.tile([128, OR], f32, tag="rsc", name="rsc")
            nc.vector.tensor_tensor(rsc, grootb_sb[:, :], rrs, ALU.mult)
            rmsh = sp.tile([128, OR], f32, tag="rmsh", name="rmsh")
            nc.vector.tensor_tensor(rmsh, mean_t, rsc, ALU.mult)
            rsh = sp.tile([128, OR], f32, tag="rsh", name="rsh")
            nc.vector.tensor_tensor(rsh, berootb_sb[:, :], rmsh,
                                    ALU.subtract)
            # y = rt * scale + shift (scale/shift broadcast over chunks)
            sc_b = rsc[:, :].unsqueeze(1).broadcast_to([128, NCH, OR])
            sh_b = rsh[:, :].unsqueeze(1).broadcast_to([128, NCH, OR])
            rt_cj = rt[:, :].rearrange("p (c j) -> p c j", c=NCH)
            tmp = bp.tile([128, NCH * OR], bf16, tag="tmp", name="tmpn")
            nc.vector.tensor_tensor(
                tmp[:, :].rearrange("p (c j) -> p c j", c=NCH),
                rt_cj, sc_b, ALU.mult)
            ysb = bp.tile([128, NCH * OR], f32, tag="ysb", name="ysb")
            nc.vector.tensor_tensor(
                ysb[:, :].rearrange("p (c j) -> p c j", c=NCH),
                tmp[:, :].rearrange("p (c j) -> p c j", c=NCH),
                sh_b, ALU.add)
            nc.gpsimd.dma_start(out=y, in_=ysb[:, :])

    nc.compile()
    return nc


def _prep_in_maps(inputs):
    """Host-side sharding + layout prep (incl. bf16 cast). 8 in_maps."""
    f = np.float32
    x_leaf = np.asarray(inputs["x_leaf"], dtype=f)
    x_mid = np.asarray(inputs["x_mid"], dtype=f)
    x_root = np.asarray(inputs["x_root"], dtype=f)
    W_leaf = np.asarray(inputs["W_leaf"], dtype=f)
    b_leaf = np.asarray(inputs["b_leaf"], dtype=f)
    g_leaf = np.asarray(inputs["g_leaf"], dtype=f)
    be_leaf = np.asarray(inputs["be_leaf"], dtype=f)
    W_mid = np.asarray(inputs["W_mid"], dtype=f)
    b_mid = np.asarray(inputs["b_mid"], dtype=f)
    g_mid = np.asarray(inputs["g_mid"], dtype=f)
    be_mid = np.asarray(inputs["be_mid"], dtype=f)
    W_root = np.asarray(inputs["W_root"], dtype=f)
    b_root = np.asarray(inputs["b_root"], dtype=f)
    g_root = np.asarray(inputs["g_root"], dtype=f)
    be_root = np.asarray(inputs["be_root"], dtype=f)

    # gene-major leaf inputs, 4 leaves stacked per 128-partition group
    xleafT = np.ascontiguousarray(
        x_leaf.reshape(NCORES, GPC, 4, B, GL).transpose(0, 1, 2, 4, 3)
        .reshape(NCORES, GPC, 128, B)).astype(NPBF16)
    # mid gene inputs: per core, mid pairs (0,1) and (2,3) stacked to 128
    xmidT = (x_mid.reshape(NCORES, 2, 2, B, GM).transpose(0, 1, 2, 4, 3)
             .reshape(NCORES, 2, 128, B)).astype(NPBF16)
    xrootT = np.ascontiguousarray(x_root.T).astype(NPBF16)     # [128, B]

    in_maps = []
    for c in range(NCORES):
        d = {}
        d["xleaf"] = np.ascontiguousarray(xleafT[c])
        # block-diagonal leaf weights [128, 16*80]
        wl = np.zeros((128, GPC * 80), f)
        for gi in range(GPC):
            for j in range(4):
                s = LPC * c + 4 * gi + j
                wl[32 * j:32 * j + 32,
                   80 * gi + 20 * j:80 * gi + 20 * j + 20] = W_leaf[s]
        d["wleaf"] = wl.astype(NPBF16)
        for src, name in ((b_leaf, "bleaf"), (g_leaf, "gleaf"),
                          (be_leaf, "beleaf")):
            d[name] = np.ascontiguousarray(
                src[LPC * c:LPC * (c + 1)].reshape(GPC, 80).T)
        d["xmid2"] = np.ascontiguousarray(xmidT[c])
        wg = np.zeros((80, GPC * 80), f)
        # gene blocks for mid pairs: [128, 2*80]
        wx2 = np.zeros((128, 2 * 80), f)
        for mi in range(MPC):
            m = MPC * c + mi
            for gj in range(4):
                idx = 4 * mi + gj
                wg[:, 80 * idx + 20 * mi:80 * idx + 20 * mi + 20] = \
                    W_mid[m, GM + 80 * gj:GM + 80 * gj + 80, :]
            pr, sub = mi // 2, mi % 2
            wx2[64 * sub:64 * sub + 64,
                80 * pr + 20 * mi:80 * pr + 20 * mi + 20] = W_mid[m, :GM, :]
        d["wgmid"] = wg.astype(NPBF16)
        d["wxmid2"] = wx2.astype(NPBF16)
        for src, name in ((b_mid, "bmid"), (g_mid, "gmid"), (be_mid, "bemid")):
            d[name] = np.ascontiguousarray(
                src[MPC * c:MPC * (c + 1)].reshape(80, 1))
        d["wcroot"] = np.ascontiguousarray(
            W_root[GR + 80 * c:GR + 80 * (c + 1), :]).astype(NPBF16)
        d["wgroot"] = np.ascontiguousarray(
            W_root[16 * c:16 * (c + 1), :]).astype(NPBF16)
        x17 = np.ones((17, B), f)
        x17[0:16, :] = xrootT[16 * c:16 * (c + 1), :].astype(f)
        d["xroot17"] = x17.astype(NPBF16)
        d["broot8"] = np.ascontiguousarray(
            (b_root / NCORES).reshape(1, OR))
        d["grootb"] = np.ascontiguousarray(
            np.broadcast_to(g_root.reshape(1, OR), (128, OR)))
        d["berootb"] = np.ascontiguousarray(
            np.broadcast_to(be_root.reshape(1, OR), (128, OR)))
        in_maps.append(d)
    return in_maps


_NC_CACHE = {}


def _get_nc():
    if "nc" not in _NC_CACHE:
        _NC_CACHE["nc"] = _build_nc()
    return _NC_CACHE["nc"]


def _postprocess(y_dev) -> np.ndarray:
    """[128, 16*38] wire-layout device output -> [2048, 38] float32."""
    out = np.asarray(y_dev, dtype=np.float32).reshape(128, NCH, OR)
    return np.ascontiguousarray(out.transpose(1, 0, 2).reshape(B, OR))


def kernel(**inputs) -> np.ndarray:
    nc = _get_nc()
    in_maps = _prep_in_maps(inputs)
    res = bass_utils.run_bass_kernel_spmd(
        nc, in_maps, core_ids=list(range(NCORES)))
    return _postprocess(res.results[0]["y"])
